# revision 36
# baseline (speedup 1.0000x reference)
"""Sparse-attention kernel for Trainium2 (8 NeuronCores, SPMD).

Math: the reference's softmax is over a singleton axis, so attention
weights are all 1.0 and the output is

    c_t = e_t * sum_{s=w_start}^{w_end} h_s[s, :]        # [1, 1024]

where the window [w_start, w_end] (<=129 rows) comes from a tiny MLP:
    p   = tanh(h_t @ fc1_w.T + fc1_b)
    p_t = S * sigmoid(p @ fc2_w.T + fc2_b)
    w_start = clip(ceil(p_t - 64), 0, None); w_end = clip(floor(p_t + 64), None, S-1)
    e_t = exp((S - p_t) / 2048)

Distribution: column-shard source_hiddens over the 8 cores
([65536, 128] each); MLP params + target are replicated.  Every core
computes p_t itself, reads ONLY a 136-row window of its shard via a
register-offset dynamic DMA, does an exact masked sum, and writes its
128 output columns.  No collectives; the host concatenates.

sigmoid is computed as (1 + tanh(z/2))/2 — the tanh activation table
is ~4 ULP vs sigmoid's 40 and exp's 400, and the integer window bounds
ceil/floor(p_t +- 64) make p_t precision the only accuracy risk.
"""

from contextlib import ExitStack

import numpy as np

import concourse.bass as bass
import concourse.mybir as mybir
from concourse.bass_utils import run_bass_kernel_spmd

S = 65536
H = 1024
NI = 256  # fc1 intermediate
NCORES = 8
HSH = H // NCORES  # 128 hidden cols per core

WIN = 136          # rows fetched (covers the <=129-row window with slack)
WP = WIN // 2      # 68 partitions x 2 rows each
BASE_MAX = S - WIN # 65400

F32 = mybir.dt.float32
F32R = mybir.dt.float32r  # fp22-truncated matmul inputs: 1-pass PE instead of 4
I32 = mybir.dt.int32
AF = mybir.ActivationFunctionType
OP = mybir.AluOpType

# The masked window sum tolerates fp22 (rel ~6e-5 << the 2e-2 gate); the
# MLP path stays true fp32 (p_t decides integer window bounds).
USE_F32R_CTX = True

# packed small-param tensor columns (partition-major layouts)
#   0..7  : ht8[p, k]  = h_t[128k + p]
#   8..9  : b1v[p, j]  = fc1_b[128j + p]
#   10..11: w2v[p, j]  = fc2_w[128j + p]
#   12    : [0,12] = fc2_b/2     (b2 half, added twice via accum over 2 cols)
#   13    : [0,13] = 32.0        (bias for e_t = exp(32 - p_t/2048))
SMALLC = 16

# w1 split: chunk0 = k-chunks 0..5, chunk1 = k-chunks 6..7
W1SPLIT = 6 * NI


def build(with_dbg=False):
    nc = bass.Bass(target_bir_lowering=False, debug=False)

    FW = F32R if USE_F32R_CTX else F32
    hs = nc.declare_dram_parameter("hs", [S, HSH], FW, isOutput=False)
    small = nc.declare_dram_parameter("small", [128, SMALLC], F32, isOutput=False)
    w1v = nc.declare_dram_parameter("w1v", [128, 8 * NI], F32, isOutput=False)
    out = nc.declare_dram_parameter("out", [1, HSH], F32, isOutput=True)
    dbgo = (
        nc.declare_dram_parameter("dbg", [1, 16], F32, isOutput=True)
        if with_dbg else None
    )

    ctx = ExitStack()
    sb = lambda name, shape, dt=F32: ctx.enter_context(nc.sbuf_tensor(name, shape, dt))
    ps = lambda name, shape, dt=F32: ctx.enter_context(nc.psum_tensor(name, shape, dt))
    sem = lambda name: ctx.enter_context(nc.semaphore(name))

    with ctx:
        w1_sb = sb("w1_sb", [128, 8 * NI])
        small_sb = sb("small_sb", [128, SMALLC])
        p2_sb = sb("p2_sb", [128, 2])
        dbg = sb("dbg_sb", [1, 16])
        ints = sb("ints_sb", [1, 4], I32)
        onesr_sb = sb("onesr_sb", [1, 128])
        junk_sb = sb("junk_sb", [1, 1])
        iota64_f = sb("iota64_f", [WP, 2])
        iotam_f = sb("iotam_f", [WP, 2])
        m1_sb = sb("m1_sb", [WP, 2])
        m2_sb = sb("m2_sb", [WP, 2])
        mask_sb = sb("mask_sb", [WP, 2], FW)
        qb_sb = sb("qb_sb", [128, 1])
        win_sb = sb("win_sb", [WP, 2 * HSH], FW)
        out_sb = sb("out_sb", [1, HSH])

        acc2a_ps = ps("acc2a_ps", [128, 1])
        acc2b_ps = ps("acc2b_ps", [128, 1])
        z_ps = ps("z_ps", [1, 1])
        bc_ps = ps("bc_ps", [128, 1])
        ctx_ps = ps("ctx_ps", [1, HSH])

        wsems = [sem(f"wsem{c}") for c in range(4)]  # w1 chunk DMAs (sync)
        hsem = sem("hsem")    # small params DMA (scalar)
        gsem = sem("gsem")    # gpsimd init
        msem = sem("msem")    # tensor-engine matmuls
        vsem = sem("vsem")    # vector steps
        ssem = sem("ssem")    # scalar compute steps
        dwin = sem("dwin")    # window DMA (sync)
        dout = sem("dout")    # output DMA (sync)
        ddbg = sem("ddbg")    # debug DMA (scalar)

        # vector-step indices (vsem thresholds)
        V_PT = 1
        V_OFF = 4
        V_Q = 6
        V_MASK = 9
        V_OUT = 10
        # msem thresholds
        M_FC1, M_Z, M_BC, M_CTX = 1, 2, 3, 5
        G_ALL = 4
        # dbg cols: 8 t=tanh(z/2), 9 p_t, 10 basef, 11 q, 13 e_t

        with nc.Block() as block:

            @block.sync
            def _(sync):
                # 4 chunks of 2 k-slices (256 KB) each: on HW the fp32
                # LDWEIGHTS pipeline behind the chunk arrivals.
                for c in range(4):
                    sync.dma_start(
                        out=w1_sb[:, c * 2 * NI : (c + 1) * 2 * NI],
                        in_=w1v[:, c * 2 * NI : (c + 1) * 2 * NI],
                    ).then_inc(wsems[c], 16)
                sync.wait_ge(vsem, V_OFF)
                with sync.register("offreg") as offreg:
                    sync.reg_load(offreg, ints[0:1, 3:4])
                    sync.dma_start(
                        out=win_sb[:, :],
                        in_=bass.AP(hs, offreg, [[2 * HSH, WP], [1, 2 * HSH]]),
                    ).then_inc(dwin, 16)
                sync.wait_ge(vsem, V_OUT)
                sync.dma_start(out=out[:, :], in_=out_sb[:, :]).then_inc(dout, 16)
                sync.wait_ge(dout, 16)

            @block.scalar
            def _(scalar):
                scalar.dma_start(out=small_sb[:, :], in_=small[:, :]).then_inc(hsem, 16)
                # preload the exp/tanh activation table set immediately
                # (input is the framework const-1.0 AP — no DMA dependency)
                one_ap = nc.const_aps.tensor(1.0, (1, 1))
                scalar.activation(junk_sb[:, :], one_ap, AF.Exp)
                # p = tanh(fc1 acc + b1), per column so b1 rides the bias port
                scalar.wait_ge(msem, M_FC1)
                scalar.activation(
                    p2_sb[:, 0:1], acc2a_ps[:, :], AF.Tanh,
                    bias=small_sb[:, 8:9],
                ).then_inc(ssem, 1)
                scalar.activation(
                    p2_sb[:, 1:2], acc2b_ps[:, :], AF.Tanh,
                    bias=small_sb[:, 9:10],
                ).then_inc(ssem, 1)
                # t = tanh(z/2) with z = fc2 psum + b2 (b2/2 on the bias port)
                scalar.wait_ge(msem, M_Z)
                scalar.wait_ge(gsem, G_ALL)
                scalar.activation(
                    dbg[:, 8:9], z_ps[0:1, 0:1], AF.Tanh,
                    scale=0.5, bias=small_sb[0:1, 12:13],
                ).then_inc(ssem, 1)
                scalar.wait_ge(vsem, V_PT)
                scalar.activation(
                    dbg[:, 13:14], dbg[:, 9:10], AF.Exp,
                    scale=-1.0 / 2048.0, bias=small_sb[0:1, 13:14],
                ).then_inc(ssem, 1)
                scalar.wait_ge(msem, M_BC)
                scalar.copy(qb_sb[:, :], bc_ps[:, :]).then_inc(ssem, 1)
                if with_dbg:
                    scalar.wait_ge(vsem, V_OUT)
                    scalar.wait_ge(ssem, 5)
                    scalar.dma_start(
                        out=dbgo[:, :], in_=dbg[:, :]
                    ).then_inc(ddbg, 16)
                    scalar.wait_ge(ddbg, 16)

            @block.tensor
            def _(tensor):
                # fc1: out.T orientation — weights stationary (2-pass fp32
                # LDW) instead of moving (4-pass).  Single msem inc on the
                # last matmul (PE completes in program order).
                tensor.wait_ge(hsem, 16)
                for k in range(8):
                    if k % 2 == 0:
                        tensor.wait_ge(wsems[k // 2], 16)
                    for j, acc in ((0, acc2a_ps), (1, acc2b_ps)):
                        inst = tensor.matmul(
                            acc[:, :],
                            w1_sb[:, k * NI + j * 128 : k * NI + (j + 1) * 128],
                            small_sb[:, k : k + 1],
                            start=(k == 0),
                            stop=(k == 7),
                            skip_group_check=True,
                        )
                inst.then_inc(msem, 1)  # M_FC1=1
                # fc2: z (sans b2) = sum_j w2v[:,j] . p2[:,j]
                tensor.wait_ge(ssem, 2)
                tensor.matmul(
                    z_ps[:, :], small_sb[:, 10:11], p2_sb[:, 0:1],
                    start=True, stop=False,
                )
                tensor.matmul(
                    z_ps[:, :], small_sb[:, 11:12], p2_sb[:, 1:2],
                    start=False, stop=True,
                ).then_inc(msem, 1)  # M_Z=2
                tensor.wait_ge(gsem, G_ALL)
                tensor.wait_ge(vsem, V_Q)
                tensor.matmul(
                    bc_ps[:, :], onesr_sb[0:1, 0:128], dbg[0:1, 11:12],
                    start=True, stop=True,
                ).then_inc(msem, 1)  # M_BC=3
                tensor.wait_ge(dwin, 16)
                tensor.wait_ge(vsem, V_MASK)
                tensor.matmul(
                    ctx_ps[:, :], mask_sb[:, 0:1], win_sb[:, 0:HSH],
                    start=True, stop=False,
                ).then_inc(msem, 1)
                tensor.matmul(
                    ctx_ps[:, :], mask_sb[:, 1:2], win_sb[:, HSH : 2 * HSH],
                    start=False, stop=True,
                ).then_inc(msem, 1)  # M_CTX=5

            @block.vector
            def _(vector):
                vn = [0]

                def step(inst):
                    inst.then_inc(vsem, 1)
                    vn[0] += 1

                def chain():
                    if vn[0]:
                        vector.wait_ge(vsem, vn[0])

                vector.wait_ge(gsem, G_ALL)
                vector.wait_ge(ssem, 3)
                step(vector.tensor_scalar(
                    dbg[:, 9:10], dbg[:, 8:9], 32768.0, 32768.0,
                    OP.mult, OP.add))  # V_PT=1: p_t = 32768 tanh(z/2) + 32768
                chain()
                step(vector.tensor_scalar(
                    ints[:, 0:1], dbg[:, 9:10], 67.0, 0.0,
                    OP.subtract, OP.max))  # v2: fused f32->i32 cast + base lo-clip
                chain()
                step(vector.tensor_scalar(
                    ints[:, 2:3], ints[:, 0:1], BASE_MAX, None, OP.min))  # v3 base
                chain()
                step(vector.tensor_scalar(
                    ints[:, 3:4], ints[:, 2:3], HSH, None, OP.mult))  # V_OFF=4
                chain()
                step(vector.tensor_copy(dbg[:, 10:11], ints[:, 2:3]))  # v5 basef
                chain()
                step(vector.tensor_tensor(
                    dbg[:, 11:12], dbg[:, 9:10], dbg[:, 10:11],
                    OP.subtract))  # V_Q=6: q = p_t - base
                vector.wait_ge(ssem, 5)
                step(vector.tensor_scalar(
                    m1_sb[:, :], iota64_f[:, :], qb_sb[0:WP, 0:1], None,
                    OP.is_ge))  # v7: r+64 >= q
                chain()
                step(vector.tensor_scalar(
                    m2_sb[:, :], iotam_f[:, :], qb_sb[0:WP, 0:1], None,
                    OP.is_le))  # v8: r-64 <= q
                chain()
                step(vector.tensor_tensor(
                    mask_sb[:, :], m1_sb[:, :], m2_sb[:, :], OP.mult))  # V_MASK=9
                vector.wait_ge(msem, M_CTX)
                vector.wait_ge(ssem, 4)
                step(vector.tensor_scalar(
                    out_sb[:, :], ctx_ps[:, :], dbg[0:1, 13:14], None,
                    OP.mult))  # V_OUT=10

            @block.gpsimd
            def _(gpsimd):
                # f32 iotas directly — values are small ints, exact in f32
                gpsimd.iota(
                    iota64_f[:, :], [[1, 2]], base=64, channel_multiplier=2,
                    allow_small_or_imprecise_dtypes=True,
                ).then_inc(gsem, 1)
                gpsimd.iota(
                    iotam_f[:, :], [[1, 2]], base=-64, channel_multiplier=2,
                    allow_small_or_imprecise_dtypes=True,
                ).then_inc(gsem, 1)
                gpsimd.memset(onesr_sb[:, :], 1.0).then_inc(gsem, 1)
                gpsimd.memset(dbg[:, :], 0.0).then_inc(gsem, 1)

    return nc


def shard_inputs(source_hiddens, target_hidden, fc1_w, fc1_b, fc2_w, fc2_b):
    hs = np.asarray(source_hiddens, dtype=np.float32)
    ht = np.asarray(target_hidden, dtype=np.float32).reshape(H)
    w1 = np.asarray(fc1_w, dtype=np.float32)
    b1 = np.asarray(fc1_b, dtype=np.float32).reshape(NI)
    w2 = np.asarray(fc2_w, dtype=np.float32).reshape(NI)
    b2 = np.asarray(fc2_b, dtype=np.float32).reshape(())

    small = np.zeros((128, SMALLC), dtype=np.float32)
    small[:, 0:8] = ht.reshape(8, 128).T
    small[:, 8:10] = b1.reshape(2, 128).T
    small[:, 10:12] = w2.reshape(2, 128).T
    small[0, 12] = np.float32(b2) / np.float32(2.0)
    small[0, 13] = 32.0

    # w1v[p, k*256 + j*128 + m] = fc1_w[j*128 + m, k*128 + p]
    w1vh = np.ascontiguousarray(
        w1.T.reshape(8, 128, 2, 128).transpose(1, 0, 2, 3).reshape(128, 8 * NI)
    )
    common = {"small": small, "w1v": w1vh}
    in_maps = []
    for i in range(NCORES):
        shard = np.ascontiguousarray(hs[:, i * HSH : (i + 1) * HSH])
        in_maps.append({"hs": shard, **common})
    return in_maps


_NC_CACHE = {}


def _get_nc(with_dbg=False):
    if with_dbg not in _NC_CACHE:
        _NC_CACHE[with_dbg] = build(with_dbg)
    return _NC_CACHE[with_dbg]


def run(in_maps, trace=False, with_dbg=False):
    nc = _get_nc(with_dbg)
    return run_bass_kernel_spmd(nc, in_maps, core_ids=list(range(NCORES)), trace=trace)


def kernel(
    source_hiddens,
    target_hidden,
    fc1_w,
    fc1_b,
    fc2_w,
    fc2_b,
    source_sentence_length,
):
    assert int(source_sentence_length) == S
    in_maps = shard_inputs(
        source_hiddens, target_hidden, fc1_w, fc1_b, fc2_w, fc2_b
    )
    res = run(in_maps, trace=False)
    return np.concatenate(
        [np.asarray(res.results[i]["out"]) for i in range(NCORES)], axis=1
    )
